# revision 1
# baseline (speedup 1.0000x reference)
"""Edge-parallel ExtractorMLP (gather + 3-layer MLP) for 8 TRN2 NeuronCores.

Strategy (pure edge parallelism, no cross-core communication):
  - 800K edges are split 100K per core.
  - The full embedding table is replicated per core as emb.T in fp16
    ([128 hidden partitions x 50000 nodes], 100KB/partition) and kept
    SBUF-resident, so both per-edge endpoint gathers are on-chip GPSIMD
    indirect_copy column gathers (hidden on partitions, edges on the free
    dim) - no transpose is needed anywhere and HBM sees each embedding
    byte exactly once.
  - The GPSIMD indirect-copy ucode only addresses data blocks up to
    ~16KB per partition, so the node axis is split into NCHUNK ranges
    (<=8192 nodes each) and each core's edges are bucketed by their
    (col_chunk, row_chunk) pair. Each 512-edge tile belongs to one
    bucket, so its two gathers read fixed table slices with chunk-local
    uint16 indices. Edge order is restored on the host afterwards.
  - The MLP runs per 512-edge tile on the tensor engine in fp16 with
    fp32 PSUM accumulation: layer 1 as 4 M-chunks x 2 K-chunks (K-chunk
    0 is the col gather, chunk 1 the row gather), layer 2 as 4 K-chunks,
    layer 3 as a single [128,1] stationary matmul. Bias+ReLU epilogues
    are split between the scalar (ACT) and vector (DVE) engines.
"""

from contextlib import ExitStack

import numpy as np

import concourse.bacc as bacc
import concourse.tile as tile
from concourse import mybir
from concourse.bass_utils import run_bass_kernel_spmd

P = 128
N = 512            # edges per tile (one fp32 PSUM bank)
IDXW = N // 16     # wrapped-index columns per tile
N_CORES = 8
N_NODES = 50000
N_EDGES = 800000
E_CORE = N_EDGES // N_CORES
NCHUNK = 7         # node-axis chunks; max chunk size must stay <= 8192
GGRP = 2           # tiles per gather (indirect_copy caps at 1024 indices)

F16 = mybir.dt.float16
F32 = mybir.dt.float32
U16 = mybir.dt.uint16

_BOUNDS = np.linspace(0, N_NODES, NCHUNK + 1).astype(np.int64)


def _build_kernel(tile_map: tuple, n_nodes: int):
    """tile_map: per-tile (col_chunk, row_chunk) ids, compile-time."""
    nc = bacc.Bacc("TRN2", target_bir_lowering=False, debug=False)
    n_tiles = len(tile_map)

    tbl = nc.dram_tensor("tbl", [P, n_nodes], F16, kind="ExternalInput")
    colw = nc.dram_tensor("colw", [P, n_tiles * IDXW], U16, kind="ExternalInput")
    roww = nc.dram_tensor("roww", [P, n_tiles * IDXW], U16, kind="ExternalInput")
    w1 = nc.dram_tensor("w1", [P, 1024], F16, kind="ExternalInput")
    w2 = nc.dram_tensor("w2", [P, 512], F16, kind="ExternalInput")
    w3 = nc.dram_tensor("w3", [P, 1], F16, kind="ExternalInput")
    b1 = nc.dram_tensor("b1", [P, 4], F32, kind="ExternalInput")
    b2 = nc.dram_tensor("b2", [P, 1], F32, kind="ExternalInput")
    b3 = nc.dram_tensor("b3", [1, 1], F32, kind="ExternalInput")
    out = nc.dram_tensor("out", [n_tiles, N], F32, kind="ExternalOutput")

    Relu = mybir.ActivationFunctionType.Relu
    Identity = mybir.ActivationFunctionType.Identity

    with tile.TileContext(nc) as tc, ExitStack() as ctx:
        tblp = ctx.enter_context(tc.tile_pool(name="tblp", bufs=1))
        idxp = ctx.enter_context(tc.tile_pool(name="idxp", bufs=1))
        wp = ctx.enter_context(tc.tile_pool(name="wp", bufs=1))
        gcp = ctx.enter_context(tc.tile_pool(name="gcp", bufs=4))
        grp = ctx.enter_context(tc.tile_pool(name="grp", bufs=4))
        x1p = ctx.enter_context(tc.tile_pool(name="x1p", bufs=12))
        x2p = ctx.enter_context(tc.tile_pool(name="x2p", bufs=4))
        op = ctx.enter_context(tc.tile_pool(name="op", bufs=8))
        pl1 = ctx.enter_context(tc.tile_pool(name="pl1", bufs=5, space="PSUM"))
        pl2 = ctx.enter_context(tc.tile_pool(name="pl2", bufs=2, space="PSUM"))
        pl3 = ctx.enter_context(tc.tile_pool(name="pl3", bufs=1, space="PSUM"))

        # ---- one-time loads -------------------------------------------
        tbl_sb = tblp.tile([P, n_nodes], F16)
        n_dma = 8
        cs = (n_nodes + n_dma - 1) // n_dma
        for c in range(n_dma):
            lo, hi = c * cs, min((c + 1) * cs, n_nodes)
            if lo >= hi:
                break
            nc.sync.dma_start(tbl_sb[:, lo:hi], tbl[:, lo:hi])

        colw_sb = idxp.tile([P, n_tiles * IDXW], U16)
        roww_sb = idxp.tile([P, n_tiles * IDXW], U16)
        nc.scalar.dma_start(colw_sb[:], colw[:])
        nc.scalar.dma_start(roww_sb[:], roww[:])

        w1_sb = wp.tile([P, 1024], F16)
        w2_sb = wp.tile([P, 512], F16)
        w3_sb = wp.tile([P, 1], F16)
        b1_sb = wp.tile([P, 4], F32)
        b2_sb = wp.tile([P, 1], F32)
        b3_sb = wp.tile([1, 1], F32)
        nc.scalar.dma_start(w1_sb[:], w1[:])
        nc.scalar.dma_start(w2_sb[:], w2[:])
        nc.scalar.dma_start(w3_sb[:], w3[:])
        nc.scalar.dma_start(b1_sb[:], b1[:])
        nc.scalar.dma_start(b2_sb[:], b2[:])
        nc.scalar.dma_start(b3_sb[:], b3[:])

        bounds = [int(b) for b in _BOUNDS]

        # gather groups: runs of up to GGRP tiles within one bucket
        groups = []
        t = 0
        while t < n_tiles:
            g = 1
            while (g < GGRP and t + g < n_tiles
                   and tile_map[t + g] == tile_map[t]):
                g += 1
            groups.append((t, g))
            t += g

        # ---- steady state ---------------------------------------------
        for t0, gsz in groups:
            c1, c2 = tile_map[t0]
            isl = slice(t0 * IDXW, (t0 + gsz) * IDXW)
            g_col = gcp.tile([P, GGRP * N], F16, tag="gcol")
            nc.gpsimd.indirect_copy(
                g_col[:, :gsz * N], data=tbl_sb[:, bounds[c1]:bounds[c1 + 1]],
                idxs=colw_sb[:, isl],
                i_know_ap_gather_is_preferred=True,
            )
            g_row = grp.tile([P, GGRP * N], F16, tag="grow")
            nc.gpsimd.indirect_copy(
                g_row[:, :gsz * N], data=tbl_sb[:, bounds[c2]:bounds[c2 + 1]],
                idxs=roww_sb[:, isl],
                i_know_ap_gather_is_preferred=True,
            )

            for j in range(gsz):
                t = t0 + j
                act_first = (t % 2 == 0)
                jsl = slice(j * N, (j + 1) * N)

                # layer 1: [E,256] @ [256,512]; K-chunk 0 = col, 1 = row
                x1s = []
                for m in range(4):
                    p1 = pl1.tile([P, N], F32, tag="pl1")
                    nc.tensor.matmul(
                        p1[:], lhsT=w1_sb[:, m * 128:(m + 1) * 128],
                        rhs=g_col[:, jsl], start=True, stop=False,
                    )
                    nc.tensor.matmul(
                        p1[:], lhsT=w1_sb[:, 512 + m * 128: 512 + (m + 1) * 128],
                        rhs=g_row[:, jsl], start=False, stop=True,
                    )
                    x1 = x1p.tile([P, N], F16, tag="x1")
                    if (m < 2) == act_first:
                        nc.scalar.activation(
                            x1[:], p1[:], Relu, bias=b1_sb[:, m:m + 1]
                        )
                    else:
                        nc.vector.tensor_scalar(
                            out=x1[:], in0=p1[:],
                            scalar1=b1_sb[:, m:m + 1], scalar2=0.0,
                            op0=mybir.AluOpType.add, op1=mybir.AluOpType.max,
                        )
                    x1s.append(x1)

                # layer 2: [E,512] @ [512,128]
                p2 = pl2.tile([P, N], F32, tag="pl2")
                for k in range(4):
                    nc.tensor.matmul(
                        p2[:], lhsT=w2_sb[:, k * 128:(k + 1) * 128],
                        rhs=x1s[k][:], start=(k == 0), stop=(k == 3),
                    )
                x2 = x2p.tile([P, N], F16, tag="x2")
                if act_first:
                    nc.scalar.activation(x2[:], p2[:], Relu, bias=b2_sb[:, 0:1])
                else:
                    nc.vector.tensor_scalar(
                        out=x2[:], in0=p2[:],
                        scalar1=b2_sb[:, 0:1], scalar2=0.0,
                        op0=mybir.AluOpType.add, op1=mybir.AluOpType.max,
                    )

                # layer 3: [E,128] @ [128,1]
                p3 = pl3.tile([P, N], F32, tag="pl3")
                nc.tensor.matmul(p3[:1, :], lhsT=w3_sb[:], rhs=x2[:],
                                 start=True, stop=True)
                o = op.tile([1, N], F32, tag="o")
                if act_first:
                    nc.vector.tensor_scalar(
                        out=o[:1, :], in0=p3[:1, :], scalar1=b3_sb[:1, 0:1],
                        scalar2=None, op0=mybir.AluOpType.add,
                    )
                else:
                    nc.scalar.activation(o[:1, :], p3[:1, :], Identity,
                                         bias=b3_sb[:1, 0:1])
                nc.sync.dma_start(out[t:t + 1, :], o[:])

    nc.compile()
    return nc


def _wrap_indices(idx: np.ndarray) -> np.ndarray:
    """[n_tiles*512] local ids -> [128, n_tiles*32] uint16 wrapped layout.

    indirect_copy unwraps each 16-partition group as
    rearrange("p s -> (s p)"), so index j of tile t sits at
    [16g + j%16, t*32 + j//16], replicated over the 8 groups g.
    """
    n_tiles = idx.shape[0] // N
    w = idx.astype(np.uint16).reshape(n_tiles, IDXW, 16).transpose(0, 2, 1)
    w = np.tile(w, (1, 8, 1))
    return np.ascontiguousarray(w.transpose(1, 0, 2).reshape(P, n_tiles * IDXW))


def _bucketize(edge_index):
    """Bucket each core's edges by (col_chunk, row_chunk).

    Returns (tile_map, per-core [col_local, row_local, slot_orig]) where
    slot_orig maps padded slot -> original edge id within the core (-1 pad).
    """
    nb = NCHUNK * NCHUNK
    cores = []
    counts = np.zeros((N_CORES, nb), np.int64)
    for c in range(N_CORES):
        sl = slice(c * E_CORE, (c + 1) * E_CORE)
        col = np.asarray(edge_index[0, sl], dtype=np.int64)
        row = np.asarray(edge_index[1, sl], dtype=np.int64)
        c1 = np.searchsorted(_BOUNDS[1:-1], col, side="right")
        c2 = np.searchsorted(_BOUNDS[1:-1], row, side="right")
        key = c1 * NCHUNK + c2
        order = np.argsort(key, kind="stable")
        counts[c] = np.bincount(key, minlength=nb)
        cores.append((col, row, key, order))

    tiles_per_bucket = np.ceil(counts.max(axis=0) / N).astype(np.int64)
    tile_map = []
    bucket_tile_start = np.zeros(nb, np.int64)
    for k in range(nb):
        bucket_tile_start[k] = len(tile_map)
        tile_map.extend([(k // NCHUNK, k % NCHUNK)] * int(tiles_per_bucket[k]))
    n_tiles = len(tile_map)

    per_core = []
    for c in range(N_CORES):
        col, row, key, order = cores[c]
        col_l = np.zeros(n_tiles * N, np.int64)
        row_l = np.zeros(n_tiles * N, np.int64)
        slot_orig = np.full(n_tiles * N, -1, np.int64)
        pos = 0
        for k in range(nb):
            nk = int(counts[c, k])
            if nk == 0:
                continue
            eids = order[pos:pos + nk]
            pos += nk
            base = int(bucket_tile_start[k]) * N
            c1, c2 = k // NCHUNK, k % NCHUNK
            col_l[base:base + nk] = col[eids] - _BOUNDS[c1]
            row_l[base:base + nk] = row[eids] - _BOUNDS[c2]
            slot_orig[base:base + nk] = eids
        per_core.append((col_l, row_l, slot_orig))
    return tuple(tile_map), per_core


def _prep_shared(emb, W1, b1, W2, b2, W3, b3):
    return {
        "tbl": np.ascontiguousarray(emb.astype(np.float16).T),
        "w1": np.ascontiguousarray(
            np.concatenate([W1[:128, :], W1[128:, :]], axis=1)
        ).astype(np.float16),
        "w2": np.ascontiguousarray(
            np.concatenate([W2[k * 128:(k + 1) * 128, :] for k in range(4)],
                           axis=1)
        ).astype(np.float16),
        "w3": W3.astype(np.float16),
        "b1": np.ascontiguousarray(b1.reshape(4, 128).T).astype(np.float32),
        "b2": b2[:, None].astype(np.float32),
        "b3": b3[None, :].astype(np.float32),
    }


_NC_CACHE = {}


def _get_nc(tile_map):
    key = (tile_map, N_NODES)
    if key not in _NC_CACHE:
        _NC_CACHE[key] = _build_kernel(tile_map, N_NODES)
    return _NC_CACHE[key]


def run(inputs: dict, trace: bool = False):
    """Run the kernel on 8 cores; returns (out [800000,1] f32, results)."""
    emb = np.asarray(inputs["emb"], dtype=np.float32)
    edge_index = np.asarray(inputs["edge_index"])
    shared = _prep_shared(
        emb,
        *[np.asarray(inputs[k], dtype=np.float32)
          for k in ("W1", "b1", "W2", "b2", "W3", "b3")]
    )
    tile_map, per_core = _bucketize(edge_index)
    in_maps = [
        dict(shared, colw=_wrap_indices(col_l), roww=_wrap_indices(row_l))
        for (col_l, row_l, _) in per_core
    ]
    nc = _get_nc(tile_map)
    res = run_bass_kernel_spmd(nc, in_maps, list(range(N_CORES)), trace=trace)
    out = np.empty((N_EDGES,), np.float32)
    for c in range(N_CORES):
        flat = res.results[c]["out"].reshape(-1)
        slot_orig = per_core[c][2]
        valid = slot_orig >= 0
        core_out = np.empty((E_CORE,), np.float32)
        core_out[slot_orig[valid]] = flat[valid]
        out[c * E_CORE:(c + 1) * E_CORE] = core_out
    return out[:, None], res


def kernel(**inputs) -> np.ndarray:
    out, _ = run(inputs, trace=False)
    return out



# revision 5
# speedup vs baseline: 4.8545x; 4.8545x over previous
"""Edge-parallel ExtractorMLP (gather + 3-layer MLP) for 8 TRN2 NeuronCores.

Strategy (pure edge parallelism, no cross-core communication):
  - All 800K edges are sorted globally by (row_half, col) and dealt
    round-robin to the 8 cores, so every core's tile t draws its edges
    from the same 4096-edge window of the global sort. Tile metadata
    (row table half, col chunk window) is therefore identical across
    cores and can be baked into the single SPMD program.
  - COL endpoint: because cols are sorted, a 512-edge tile's cols span
    ~512 consecutive nodes (~5 aligned 128-node chunks). The gather is
    done ON THE TENSOR ENGINE as one-hot matmuls: a node-major copy of
    the embedding table lives in SBUF ([128 node partitions x 391
    chunks x 128 features]); for each chunk a [128, 512] 0/1 selection
    matrix S (built by the vector engine from DMA-broadcast col values
    via subtract+is_equal against a per-partition iota) is multiplied
    against the chunk to accumulate emb[col] in PSUM - exact, and it
    rides otherwise-idle PE/DVE cycles.
  - ROW endpoint: rows are random, so they use SWDGE dma_gather
    (transpose=True) from the HBM [50000, 128] fp16 table: 512 indices
    per tile, ~9.3ns/descriptor of Q7 time - the pacing engine. Rows
    are int16 per dma_gather's ABI, hence the row_half split (<32768
    nodes per half, half-local indices).
  - The MLP runs per 512-edge tile on the tensor engine in fp16 with
    fp32 PSUM accumulation: layer 1 as 4 M-chunks x 2 K-chunks (K-chunk
    0 is the one-hot col gather, chunk 1 the row gather), layer 2 as 4
    K-chunks, layer 3 as a single [128,1] stationary matmul. Bias+ReLU
    epilogues are split between the scalar (ACT) and vector (DVE)
    engines; col-value broadcasts and S-builds are software-pipelined
    one to two tiles ahead so no engine queue blocks another.
  - Edge order is restored on the host afterwards.
"""

from contextlib import ExitStack

import numpy as np

import concourse.bacc as bacc
import concourse.tile as tile
from concourse import mybir
from concourse.bass_utils import run_bass_kernel_spmd

P = 128
N = 512            # edges per tile (one fp32 PSUM bank)
IDXW = N // 16     # wrapped-index columns per tile
N_CORES = 8
GT = N * N_CORES   # global edges per tile row (4096)
N_NODES = 50000
N_NODES_PAD = 50048  # 391 chunks of 128
NCH_TBL = N_NODES_PAD // 128
N_EDGES = 800000
E_CORE = N_EDGES // N_CORES
HALF = 25000       # row table half size (int16 dma_gather indices)

F16 = mybir.dt.float16
F32 = mybir.dt.float32
I16 = mybir.dt.int16


def _build_kernel(tiles_meta: tuple):
    """tiles_meta: per-tile (row_half, col_chunk_lo, n_chunks), compile-time."""
    nc = bacc.Bacc("TRN2", target_bir_lowering=False, debug=False)
    n_tiles = len(tiles_meta)

    tblrow = nc.dram_tensor("tblrow", [N_NODES, P], F16, kind="ExternalInput")
    tblnm = nc.dram_tensor("tblnm", [P, NCH_TBL * 128], F16, kind="ExternalInput")
    roww = nc.dram_tensor("roww", [P, n_tiles * IDXW], I16, kind="ExternalInput")
    colloc = nc.dram_tensor("colloc", [1, n_tiles * N], F16, kind="ExternalInput")
    iota = nc.dram_tensor("iota", [P, 1], F32, kind="ExternalInput")
    w1 = nc.dram_tensor("w1", [P, 1024], F16, kind="ExternalInput")
    w2 = nc.dram_tensor("w2", [P, 512], F16, kind="ExternalInput")
    w3 = nc.dram_tensor("w3", [P, 1], F16, kind="ExternalInput")
    b1 = nc.dram_tensor("b1", [P, 4], F32, kind="ExternalInput")
    b2 = nc.dram_tensor("b2", [P, 1], F32, kind="ExternalInput")
    b3 = nc.dram_tensor("b3", [1, 1], F32, kind="ExternalInput")
    out = nc.dram_tensor("out", [n_tiles, N], F32, kind="ExternalOutput")

    Relu = mybir.ActivationFunctionType.Relu
    Identity = mybir.ActivationFunctionType.Identity
    Op = mybir.AluOpType

    with tile.TileContext(nc) as tc, ExitStack() as ctx:
        tp = ctx.enter_context(tc.tile_pool(name="tp", bufs=1))
        idxp = ctx.enter_context(tc.tile_pool(name="idxp", bufs=1))
        wp = ctx.enter_context(tc.tile_pool(name="wp", bufs=1))
        cbp = ctx.enter_context(tc.tile_pool(name="cbp", bufs=4))
        sp = ctx.enter_context(tc.tile_pool(name="sp", bufs=14))
        grp = ctx.enter_context(tc.tile_pool(name="grp", bufs=6))
        gcp = ctx.enter_context(tc.tile_pool(name="gcp", bufs=4))
        x1p = ctx.enter_context(tc.tile_pool(name="x1p", bufs=12))
        x2p = ctx.enter_context(tc.tile_pool(name="x2p", bufs=4))
        op = ctx.enter_context(tc.tile_pool(name="op", bufs=8))
        pg = ctx.enter_context(tc.tile_pool(name="pg", bufs=2, space="PSUM"))
        pl1 = ctx.enter_context(tc.tile_pool(name="pl1", bufs=4, space="PSUM"))
        pl2 = ctx.enter_context(tc.tile_pool(name="pl2", bufs=1, space="PSUM"))
        pl3 = ctx.enter_context(tc.tile_pool(name="pl3", bufs=1, space="PSUM"))

        # ---- one-time loads -------------------------------------------
        tblnm_sb = tp.tile([P, NCH_TBL * 128], F16)
        n_dma = 16
        cs = (NCH_TBL * 128 + n_dma - 1) // n_dma
        for c in range(n_dma):
            lo, hi = c * cs, min((c + 1) * cs, NCH_TBL * 128)
            nc.sync.dma_start(tblnm_sb[:, lo:hi], tblnm[:, lo:hi])

        roww_sb = idxp.tile([P, n_tiles * IDXW], I16)
        nc.scalar.dma_start(roww_sb[:], roww[:])
        iota_sb = wp.tile([P, 1], F32)
        nc.scalar.dma_start(iota_sb[:], iota[:])

        w1_sb = wp.tile([P, 1024], F16)
        w2_sb = wp.tile([P, 512], F16)
        w3_sb = wp.tile([P, 1], F16)
        b1_sb = wp.tile([P, 4], F32)
        b2_sb = wp.tile([P, 1], F32)
        b3_sb = wp.tile([1, 1], F32)
        nc.scalar.dma_start(w1_sb[:], w1[:])
        nc.scalar.dma_start(w2_sb[:], w2[:])
        nc.scalar.dma_start(w3_sb[:], w3[:])
        nc.scalar.dma_start(b1_sb[:], b1[:])
        nc.scalar.dma_start(b2_sb[:], b2[:])
        nc.scalar.dma_start(b3_sb[:], b3[:])

        # col values broadcast (scalar HWDGE) and one-hot S builds (DVE)
        # are software-pipelined ahead of their consuming tile.
        def emit_cb(t):
            cb = cbp.tile([P, N], F16, tag="cb", name=f"cb{t}")
            nc.scalar.dma_start(
                cb[:], colloc[0:1, t * N:(t + 1) * N].broadcast_to([P, N]))
            return cb

        def emit_s(t, cb):
            nch = tiles_meta[t][2]
            ss = []
            for kk in range(nch):
                s = sp.tile([P, N], F16, tag="S", name=f"s{t}_{kk}")
                nc.vector.tensor_scalar(
                    out=s[:], in0=cb[:], scalar1=iota_sb[:, 0:1],
                    scalar2=float(128 * kk),
                    op0=Op.subtract, op1=Op.is_equal,
                )
                ss.append(s)
            return ss

        cbs = {0: emit_cb(0)}
        if n_tiles > 1:
            cbs[1] = emit_cb(1)
        s_next = emit_s(0, cbs[0])

        # ---- steady state ---------------------------------------------
        for t, (rh, clo, nch) in enumerate(tiles_meta):
            # row endpoint: SWDGE gather from HBM (feature-major output)
            g_row = grp.tile([P, 1, N], F16, tag="grow")
            nc.gpsimd.dma_gather(
                g_row[:], tblrow[rh * HALF:rh * HALF + HALF, :],
                roww_sb[:, t * IDXW:(t + 1) * IDXW], N, N, P, transpose=True,
            )

            if t + 2 < n_tiles:
                cbs[t + 2] = emit_cb(t + 2)

            # col endpoint: one-hot matmuls against node-major table chunks
            s_cur = s_next
            pg_t = pg.tile([P, N], F32, tag="pg")
            for kk in range(nch):
                nc.tensor.matmul(
                    pg_t[:],
                    lhsT=tblnm_sb[:, (clo + kk) * 128:(clo + kk + 1) * 128],
                    rhs=s_cur[kk][:], start=(kk == 0), stop=(kk == nch - 1),
                )
            g_col = gcp.tile([P, N], F16, tag="gcol")
            nc.scalar.activation(g_col[:], pg_t[:], Identity)

            if t + 1 < n_tiles:
                s_next = emit_s(t + 1, cbs[t + 1])

            # layer 1: [E,256] @ [256,512]; K-chunk 0 = col, 1 = row
            x1s = []
            for m in range(4):
                p1 = pl1.tile([P, N], F32, tag="pl1")
                nc.tensor.matmul(
                    p1[:], lhsT=w1_sb[:, m * 128:(m + 1) * 128],
                    rhs=g_col[:], start=True, stop=False,
                )
                nc.tensor.matmul(
                    p1[:], lhsT=w1_sb[:, 512 + m * 128: 512 + (m + 1) * 128],
                    rhs=g_row[:, 0, :], start=False, stop=True,
                )
                x1 = x1p.tile([P, N], F16, tag="x1")
                if m < 2:
                    nc.scalar.activation(
                        x1[:], p1[:], Relu, bias=b1_sb[:, m:m + 1]
                    )
                else:
                    nc.vector.tensor_scalar(
                        out=x1[:], in0=p1[:],
                        scalar1=b1_sb[:, m:m + 1], scalar2=0.0,
                        op0=Op.add, op1=Op.max,
                    )
                x1s.append(x1)

            # layer 2: [E,512] @ [512,128]
            p2 = pl2.tile([P, N], F32, tag="pl2")
            for k in range(4):
                nc.tensor.matmul(
                    p2[:], lhsT=w2_sb[:, k * 128:(k + 1) * 128],
                    rhs=x1s[k][:], start=(k == 0), stop=(k == 3),
                )
            x2 = x2p.tile([P, N], F16, tag="x2")
            nc.scalar.activation(x2[:], p2[:], Relu, bias=b2_sb[:, 0:1])

            # layer 3: [E,128] @ [128,1]
            p3 = pl3.tile([P, N], F32, tag="pl3")
            nc.tensor.matmul(p3[:1, :], lhsT=w3_sb[:], rhs=x2[:],
                             start=True, stop=True)
            o = op.tile([1, N], F32, tag="o")
            nc.vector.tensor_scalar(
                out=o[:1, :], in0=p3[:1, :], scalar1=b3_sb[:1, 0:1],
                scalar2=None, op0=Op.add,
            )
            nc.sync.dma_start(out[t:t + 1, :], o[:])

    nc.compile()
    return nc


def _wrap_indices(idx: np.ndarray) -> np.ndarray:
    """[n_tiles*512] local ids -> [128, n_tiles*32] int16 wrapped layout.

    dma_gather unwraps each 16-partition group as
    rearrange("p s -> (s p)"), so index j of tile t sits at
    [16g + j%16, t*32 + j//16], replicated over the 8 groups g.
    """
    n_tiles = idx.shape[0] // N
    w = idx.astype(np.int16).reshape(n_tiles, IDXW, 16).transpose(0, 2, 1)
    w = np.tile(w, (1, 8, 1))
    return np.ascontiguousarray(w.transpose(1, 0, 2).reshape(P, n_tiles * IDXW))


def _plan(edge_index):
    """Global (row_half, col) sort + round-robin deal to cores.

    Returns (tiles_meta, per-core (colloc f16 [1, S], row_local i64 [S],
    slot_orig i64 [S])) with S = n_tiles*512 slots per core.
    """
    col = np.asarray(edge_index[0], dtype=np.int64)
    row = np.asarray(edge_index[1], dtype=np.int64)
    half = (row >= HALF).astype(np.int64)
    order = np.lexsort((col, half))
    scol, srow, shalf = col[order], row[order], half[order]
    b0 = int((half == 0).sum())
    bounds = [(0, b0, 0), (b0, N_EDGES, 1)]

    tiles_meta = []
    # padded global slot -> sorted-position (or -1)
    gslots = []
    for s, e, k in bounds:
        nt = -(-(e - s) // GT)
        for i in range(nt):
            p0, p1 = s + i * GT, min(s + (i + 1) * GT, e)
            wlo = int(scol[p0])
            whi = int(scol[p1 - 1])
            clo = wlo >> 7
            nch = (whi >> 7) - clo + 1
            tiles_meta.append((k, clo, nch))
            sl = np.full(GT, -1, np.int64)
            sl[:p1 - p0] = np.arange(p0, p1)
            gslots.append(sl)
    g = np.stack(gslots)                      # [n_tiles, GT]
    n_tiles = len(tiles_meta)
    g = g.reshape(n_tiles, N, N_CORES)        # [t, j, core]

    clo_arr = np.array([m[1] for m in tiles_meta], np.int64)[:, None]
    rh_arr = np.array([m[0] for m in tiles_meta], np.int64)[:, None]

    per_core = []
    for c in range(N_CORES):
        gp = g[:, :, c]                       # [t, j] sorted positions
        valid = gp >= 0
        gp_safe = np.where(valid, gp, 0)
        cl = np.where(valid, scol[gp_safe] - (clo_arr << 7), 0)
        rl = np.where(valid, srow[gp_safe] - rh_arr * HALF, 0)
        so = np.where(valid, order[gp_safe], -1)
        assert cl.max() < 2048, cl.max()
        per_core.append((
            cl.reshape(-1).astype(np.float16)[None, :],
            rl.reshape(-1),
            so.reshape(-1),
        ))
    return tuple(tiles_meta), per_core


def _prep_shared(emb, W1, b1, W2, b2, W3, b3):
    emb16 = emb.astype(np.float16)
    pad = np.zeros((N_NODES_PAD, P), np.float16)
    pad[:N_NODES] = emb16
    tblnm = np.ascontiguousarray(
        pad.reshape(NCH_TBL, 128, 128).transpose(1, 0, 2).reshape(P, -1))
    return {
        "tblrow": np.ascontiguousarray(emb16),
        "tblnm": tblnm,
        "iota": np.arange(128, dtype=np.float32)[:, None],
        "w1": np.ascontiguousarray(
            np.concatenate([W1[:128, :], W1[128:, :]], axis=1)
        ).astype(np.float16),
        "w2": np.ascontiguousarray(
            np.concatenate([W2[k * 128:(k + 1) * 128, :] for k in range(4)],
                           axis=1)
        ).astype(np.float16),
        "w3": W3.astype(np.float16),
        "b1": np.ascontiguousarray(b1.reshape(4, 128).T).astype(np.float32),
        "b2": b2[:, None].astype(np.float32),
        "b3": b3[None, :].astype(np.float32),
    }


_NC_CACHE = {}


def _get_nc(tiles_meta):
    if tiles_meta not in _NC_CACHE:
        _NC_CACHE[tiles_meta] = _build_kernel(tiles_meta)
    return _NC_CACHE[tiles_meta]


def run(inputs: dict, trace: bool = False):
    """Run the kernel on 8 cores; returns (out [800000,1] f32, results)."""
    emb = np.asarray(inputs["emb"], dtype=np.float32)
    edge_index = np.asarray(inputs["edge_index"])
    shared = _prep_shared(
        emb,
        *[np.asarray(inputs[k], dtype=np.float32)
          for k in ("W1", "b1", "W2", "b2", "W3", "b3")]
    )
    tiles_meta, per_core = _plan(edge_index)
    in_maps = [
        dict(shared, colloc=np.ascontiguousarray(cl),
             roww=_wrap_indices(rl))
        for (cl, rl, _) in per_core
    ]
    nc = _get_nc(tiles_meta)
    res = run_bass_kernel_spmd(nc, in_maps, list(range(N_CORES)), trace=trace)
    out = np.empty((N_EDGES,), np.float32)
    for c in range(N_CORES):
        flat = res.results[c]["out"].reshape(-1)
        so = per_core[c][2]
        valid = so >= 0
        out[so[valid]] = flat[valid]
    return out[:, None], res


def kernel(**inputs) -> np.ndarray:
    out, _ = run(inputs, trace=False)
    return out
